# revision 1
# baseline (speedup 1.0000x reference)
"""BatchTopK SAE encoder on 8 Trainium2 NeuronCores.

Strategy
--------
Tensor-parallel over dict_size: core c computes the encoder GEMM for dict
rows [c*4096, (c+1)*4096):

    acts_c^T [4096, 2048] = relu(W_c @ (x - b_dec)^T + b_enc_c)

as float32r (tf32-class) matmuls on the PE array — one pass, PE-bound.
The global batch top-(k*B) is then resolved on the host from the
device-computed activations: a conservative threshold screens ~0.5% of
elements as candidates, the (k*B)-th largest device value defines the cut,
and the narrow borderline band (where the f32r error could flip the
selection) is recomputed exactly in fp64 from the original fp32 inputs so
the selected set matches an exact-fp32 reference. Everything outside the
band is classified directly by its device value.

The kernel returns scatter(top-(k*B) values) as a dense [B, D_DICT] fp32
array, matching the reference semantics (ties broken by lower flat index).
"""

import sys

sys.path.insert(0, "/opt/trn_rl_repo")

import numpy as np

# ---- problem constants (from the spec; asserted at runtime) ----
B = 2048           # batch
D = 2048           # activation dim (contraction)
DD = 32768         # dict size
NCORES = 8
FSH = DD // NCORES # 4096 dict rows per core
KT = D // 128      # 16 contraction tiles
FT = FSH // 128    # 32 f-tiles per core
NB = B // 512      # 4 batch chunks of 512

_STATE = {}


def _build_nc():
    from concourse import bacc
    import concourse.mybir as mybir
    import concourse.tile as tile

    F32 = mybir.dt.float32
    F16 = mybir.dt.float16
    F32R = mybir.dt.float32r
    RELU = mybir.ActivationFunctionType.Relu

    nc = bacc.Bacc("TRN2", target_bir_lowering=False, debug=False, num_devices=NCORES)
    xt_d = nc.dram_tensor("xt", [128, KT * NB * 512], F32R, kind="ExternalInput").ap()
    wt_d = nc.dram_tensor("wt", [128, FT * KT * 128], F32R, kind="ExternalInput").ap()
    be_d = nc.dram_tensor("be", [128, FT], F32, kind="ExternalInput").ap()
    acts_d = nc.dram_tensor("acts", [FSH, B], F16, kind="ExternalOutput").ap()

    FA = 6  # phase-A f-tiles (resident W) — covers PE while x streams in

    with tile.TileContext(nc) as tc:
        with (
            tc.tile_pool(name="xres", bufs=1) as xpool,
            tc.tile_pool(name="wa", bufs=1) as wapool,
            tc.tile_pool(name="wstream", bufs=2) as wpool,
            tc.tile_pool(name="eplg", bufs=4) as opool,
            tc.tile_pool(name="ps", bufs=2, space="PSUM") as pspool,
        ):
            # DMA issue order matters: transfers complete in queue order, so
            # interleave the phase-A W tiles with the x chunks to unblock the
            # first matmul chains as early as possible.
            xts = [None] * NB
            was = [None] * FA

            def load_wa(f):
                wa = wapool.tile([128, KT * 128], F32R, tag=f"wa{f}")
                nc.sync.dma_start(
                    out=wa, in_=wt_d[:, f * KT * 128 : (f + 1) * KT * 128]
                )
                was[f] = wa

            def load_xk0(kks):
                # fine-grained per-kk tiles of the first batch chunk: the
                # first chains start as soon as the first 256KB slice lands
                if xts[0] is None:
                    xts[0] = [None] * KT
                for kk in kks:
                    xk = xpool.tile([128, 512], F32R, tag=f"xt0_{kk}")
                    nc.sync.dma_start(out=xk, in_=xt_d[:, kk * 512 : (kk + 1) * 512])
                    xts[0][kk] = xk

            def load_xnb(nb):
                xnb = xpool.tile([128, KT * 512], F32R, tag=f"xt{nb}")
                nc.sync.dma_start(
                    out=xnb, in_=xt_d[:, nb * KT * 512 : (nb + 1) * KT * 512]
                )
                xts[nb] = xnb

            be = xpool.tile([128, FT], F32, tag="be")
            load_wa(0)
            load_xk0(range(0, 3))
            load_wa(1)
            load_xk0(range(3, 9))
            load_wa(2)
            load_xk0(range(9, 16))
            nc.sync.dma_start(out=be, in_=be_d)
            load_wa(3)
            load_wa(4)
            load_wa(5)
            load_xnb(1)
            load_xnb(2)
            load_xnb(3)

            def chain(f, nb, wt):
                ps = pspool.tile([128, 512], F32, tag=f"ps{nb}")
                for kk in range(KT):
                    rhs = (
                        xts[0][kk]
                        if nb == 0
                        else xts[nb][:, kk * 512 : (kk + 1) * 512]
                    )
                    nc.tensor.matmul(
                        ps,
                        wt[:, kk * 128 : (kk + 1) * 128],
                        rhs,
                        start=(kk == 0),
                        stop=(kk == KT - 1),
                    )
                ot = opool.tile([128, 512], F16, tag="ot")
                nc.scalar.activation(ot, ps, func=RELU, bias=be[:, f : f + 1])
                nc.sync.dma_start(
                    out=acts_d[f * 128 : (f + 1) * 128, nb * 512 : (nb + 1) * 512],
                    in_=ot,
                )

            # phase A: nb-major staircase over the resident f-tiles, keeping
            # the PE busy while the rest of x is still loading
            for nb in range(NB):
                for f in range(FA):
                    chain(f, nb, was[f])

            # phase B: stream the remaining W tiles
            for f in range(FA, FT):
                wt = wpool.tile([128, KT * 128], F32R, tag="wt")
                nc.sync.dma_start(
                    out=wt, in_=wt_d[:, f * KT * 128 : (f + 1) * KT * 128]
                )
                for nb in range(NB):
                    chain(f, nb, wt)

    nc.compile()
    return nc


def _get_nc():
    if "nc" not in _STATE:
        _STATE["nc"] = _build_nc()
    return _STATE["nc"]


def _pack_x(xc):
    # xc [B, D] -> [128, NB*KT*512]: block (nb, kk) holds xc^T[kk*128+p, nb*512+bb]
    return np.ascontiguousarray(
        xc.T.reshape(KT, 128, NB, 512).transpose(1, 2, 0, 3).reshape(128, -1)
    )


def _pack_w(Wsh):
    # Wsh [FSH, D] -> [128, FT*KT*128]: block (f, kk) holds W[f*128+ff, kk*128+p]
    return np.ascontiguousarray(
        Wsh.reshape(FT, 128, KT, 128).transpose(3, 0, 2, 1).reshape(128, -1)
    )


def _get_runner():
    """Build the Bass program once and return a cached jitted SPMD callable.

    runner(xt, wt_concat, be_concat) -> actsT [DD, B] (numpy).
    xt is replicated to all 8 cores; wt/be are sharded along axis 0.
    """
    if "runner" in _STATE:
        return _STATE["runner"]

    import jax
    import jax.numpy as jnp
    from jax.sharding import Mesh, PartitionSpec
    from jax.experimental.shard_map import shard_map
    from concourse import mybir
    from concourse.bass2jax import (
        _bass_exec_p,
        install_neuronx_cc_hook,
        partition_id_tensor,
    )

    nc = _get_nc()
    install_neuronx_cc_hook()

    pname = nc.partition_id_tensor.name if nc.partition_id_tensor else None
    in_names, out_names, out_avals = [], [], []
    for alloc in nc.m.functions[0].allocations:
        if not isinstance(alloc, mybir.MemoryLocationSet):
            continue
        name = alloc.memorylocations[0].name
        if alloc.kind == "ExternalInput":
            if name != pname:
                in_names.append(name)
        elif alloc.kind == "ExternalOutput":
            out_names.append(name)
            out_avals.append(
                jax.core.ShapedArray(tuple(alloc.tensor_shape), mybir.dt.np(alloc.dtype))
            )
    assert set(in_names) == {"xt", "wt", "be"}, in_names
    assert out_names == ["acts"], out_names
    all_in_names = in_names + out_names + ([pname] if pname else [])

    def _body(*args):
        operands = list(args)
        if pname:
            operands.append(partition_id_tensor())
        outs = _bass_exec_p.bind(
            *operands,
            out_avals=tuple(out_avals),
            in_names=tuple(all_in_names),
            out_names=tuple(out_names),
            lowering_input_output_aliases=(),
            sim_require_finite=True,
            sim_require_nnan=True,
            nc=nc,
        )
        return tuple(outs)

    devices = jax.devices()[:NCORES]
    assert len(devices) == NCORES, f"need {NCORES} neuron cores, got {len(devices)}"
    mesh = Mesh(np.asarray(devices), ("core",))
    arg_names = in_names + out_names
    in_specs = tuple(
        PartitionSpec() if nm == "xt" else PartitionSpec("core") for nm in arg_names
    )
    sharded = jax.jit(
        shard_map(
            _body,
            mesh=mesh,
            in_specs=in_specs,
            out_specs=(PartitionSpec("core"),),
            check_rep=False,
        )
    )

    from jax.sharding import NamedSharding

    # device-resident zero output-init buffers, uploaded once and reused
    zeros = [
        jax.device_put(
            np.zeros((NCORES * a.shape[0], *a.shape[1:]), a.dtype),
            NamedSharding(mesh, PartitionSpec("core")),
        )
        for a in out_avals
    ]

    def runner(xt, wt_concat, be_concat):
        args = {"xt": xt, "wt": wt_concat, "be": be_concat}
        out = sharded(*[args[nm] for nm in in_names], *zeros)
        return np.asarray(out[0])  # [DD, B]

    _STATE["runner"] = runner
    return runner


def _prep_inputs(x, W_enc, b_enc, b_dec):
    xc = (x.astype(np.float32) - b_dec.astype(np.float32)[None, :]).astype(np.float32)
    xt = _pack_x(xc)
    wt_concat = np.concatenate(
        [
            _pack_w(np.ascontiguousarray(W_enc[c * FSH : (c + 1) * FSH]))
            for c in range(NCORES)
        ],
        axis=0,
    )
    be_concat = np.concatenate(
        [
            np.ascontiguousarray(
                b_enc[c * FSH : (c + 1) * FSH].astype(np.float32).reshape(FT, 128).T
            )
            for c in range(NCORES)
        ],
        axis=0,
    )
    return xt, wt_concat, be_concat


def _run_device(x, W_enc, b_enc, b_dec, trace=False, trace_kwargs=None):
    if trace:
        # profiling path via run_bass_kernel_spmd (NTFF capture)
        from concourse.bass_utils import run_bass_kernel_spmd

        nc = _get_nc()
        xc = (x.astype(np.float32) - b_dec.astype(np.float32)[None, :]).astype(
            np.float32
        )
        xt = _pack_x(xc)
        in_maps = []
        for c in range(NCORES):
            in_maps.append(
                {
                    "xt": xt,
                    "wt": _pack_w(
                        np.ascontiguousarray(W_enc[c * FSH : (c + 1) * FSH])
                    ),
                    "be": np.ascontiguousarray(
                        b_enc[c * FSH : (c + 1) * FSH]
                        .astype(np.float32)
                        .reshape(FT, 128)
                        .T
                    ),
                }
            )
        res = run_bass_kernel_spmd(
            nc, in_maps, list(range(NCORES)), trace=True, **(trace_kwargs or {})
        )
        _STATE["last_result"] = res
        return np.concatenate(
            [res.results[c]["acts"] for c in range(NCORES)], axis=0
        )

    runner = _get_runner()
    xt, wt_concat, be_concat = _prep_inputs(x, W_enc, b_enc, b_dec)
    return runner(xt, wt_concat, be_concat)


def _exact_vals(x32, W32, be64, f_idx, b_idx):
    """Accurate fp32 recompute of pre-relu acts at (b, f) pairs.

    Grouped by batch row so each group is a single BLAS sgemv — same
    accuracy class as the reference's own fp32 einsum.
    """
    n = len(f_idx)
    if n == 0:
        return np.zeros(0, np.float64)
    order = np.argsort(b_idx, kind="stable")
    fs, bs = f_idx[order], b_idx[order]
    ub, starts = np.unique(bs, return_index=True)
    ends = np.append(starts[1:], n)
    out = np.empty(n, np.float32)
    for i, b in enumerate(ub):
        s, e = starts[i], ends[i]
        out[s:e] = W32[fs[s:e]] @ x32[b]
    res = np.empty(n, np.float64)
    res[order] = out.astype(np.float64)
    return res + be64[f_idx]


def _select_topk(actsT, kb, x32, W32, be64, sigma):
    """Exact top-kb selection (reference semantics) from device f16 acts.

    Returns (b_idx, f_idx, values[fp32]) of the selected elements.
    actsT: [DD, B] float16 device activations.
    """
    DDl, Bl = actsT.shape
    total = DDl * Bl
    empty = (np.zeros(0, np.int64), np.zeros(0, np.int64), np.zeros(0, np.float32))
    if kb <= 0:
        return empty
    kb = min(kb, total)

    # abs error bound of device f16 acts vs exact fp32:
    #   f32r GEMM err (~1.6e-3*sigma) + f16 quantization near tau (~1.6e-3*sigma)
    errtot = max(3.2e-3 * sigma, 1e-7)

    # conservative screen: comfortably more candidates than kb
    cnt = 0
    for t_frac in (2.45, 2.0, 1.5, 1.0, 0.5, 0.0):
        t_lo = t_frac * sigma
        m = actsT > np.float16(t_lo)
        cnt = int(m.sum())
        if cnt >= kb + max(1024, kb // 16) or t_frac == 0.0:
            break

    f_idx, b_idx = np.nonzero(m)
    vals = actsT[m].astype(np.float32)

    if cnt <= kb:
        # everything positive is selected (selected zeros are no-ops)
        ex = _exact_vals(x32, W32, be64, f_idx, b_idx)
        keep = ex > 0
        return (
            b_idx[keep],
            f_idx[keep],
            np.maximum(ex[keep], 0.0).astype(np.float32),
        )

    part = np.partition(vals, cnt - kb)
    tau_dev = float(part[cnt - kb])

    band = 2.5 * errtot
    for _ in range(24):
        refine = vals > tau_dev - band
        nr = int(refine.sum())
        if nr < kb:
            band *= 2.0
            continue
        fr, br = f_idx[refine], b_idx[refine]
        ex = _exact_vals(x32, W32, be64, fr, br)
        flat = br.astype(np.int64) * DDl + fr.astype(np.int64)
        # reference order: value desc, flat index asc on ties
        order = np.lexsort((flat, -ex))
        take = order[:kb]
        tau_exact = float(ex[take[-1]])
        # excluded elements have f16 <= tau_dev - band, so their exact value
        # is <= tau_dev - band + errtot; selection is airtight iff
        # tau_exact is above that.
        if tau_exact > tau_dev - band + errtot or (band > 2.0 * sigma + 1.0):
            vsel = np.maximum(ex[take], 0.0).astype(np.float32)
            return (br[take], fr[take], vsel)
        band *= 2.0
        if tau_dev - band < t_lo + errtot and t_lo > 0:
            # widen past the screen: fall back to all-positives screen
            m = actsT > np.float16(0.0)
            cnt = int(m.sum())
            f_idx, b_idx = np.nonzero(m)
            vals = actsT[m].astype(np.float32)
            t_lo = 0.0
            if cnt <= kb:
                ex = _exact_vals(x32, W32, be64, f_idx, b_idx)
                keep = ex > 0
                return (
                    b_idx[keep],
                    f_idx[keep],
                    np.maximum(ex[keep], 0.0).astype(np.float32),
                )
            part = np.partition(vals, cnt - kb)
            tau_dev = float(part[cnt - kb])
    raise RuntimeError("top-k band search failed to converge")


def _kernel_numpy_fallback(x, W_enc, b_enc, b_dec, k):
    x32 = x.astype(np.float32)
    acts = np.maximum(
        (x32 - b_dec.astype(np.float32)) @ W_enc.astype(np.float32).T
        + b_enc.astype(np.float32),
        0.0,
    )
    flat = acts.reshape(-1)
    kb = int(k) * x.shape[0]
    if kb <= 0:
        return np.zeros_like(acts)
    kb = min(kb, flat.size)
    idx = np.argpartition(flat, flat.size - kb)[flat.size - kb :]
    # exact reference tie-break: value desc, index asc
    order = np.lexsort((idx, -flat[idx].astype(np.float64)))
    idx = idx[order[:kb]]
    out = np.zeros_like(flat)
    out[idx] = flat[idx]
    return out.reshape(acts.shape)


def kernel(x, W_enc, b_enc, b_dec, k):
    x = np.asarray(x)
    W_enc = np.asarray(W_enc)
    b_enc = np.asarray(b_enc)
    b_dec = np.asarray(b_dec)
    kb = int(k) * x.shape[0]

    if x.shape != (B, D) or W_enc.shape != (DD, D):
        return _kernel_numpy_fallback(x, W_enc, b_enc, b_dec, k)

    actsT = _run_device(x, W_enc, b_enc, b_dec)  # [DD, B] f16

    if not np.all(np.isfinite(actsT[:: max(1, DD // 256)])) or np.any(
        actsT[:: max(1, DD // 256)] == np.inf
    ):
        return _kernel_numpy_fallback(x, W_enc, b_enc, b_dec, k)

    x32 = (x.astype(np.float32) - b_dec.astype(np.float32)[None, :]).astype(np.float32)
    W32 = np.ascontiguousarray(W_enc.astype(np.float32))
    be64 = b_enc.astype(np.float64)

    sub = actsT[:: max(1, DD // 1024)].astype(np.float32)
    sigma = float(np.sqrt(2.0 * np.mean(np.square(sub))))
    if not np.isfinite(sigma) or sigma <= 0:
        sigma = 1.0

    b_sel, f_sel, v_sel = _select_topk(actsT, kb, x32, W32, be64, sigma)

    out = np.zeros((B, DD), np.float32)
    out[b_sel, f_sel] = v_sel
    return out



# revision 2
# speedup vs baseline: 2.2948x; 2.2948x over previous
"""BatchTopK SAE encoder on 8 Trainium2 NeuronCores (fp8 DoubleRow GEMM).

Strategy
--------
Tensor-parallel over dict_size: core c computes the encoder GEMM for dict
rows [c*4096, (c+1)*4096):

    acts_c^T [4096, 2048] = relu((W_c*64)_fp8 @ (x - b_dec)_fp8^T / 64 + b_enc_c)

as float8e4 (e4m3) matmuls in MatmulPerfMode.DoubleRow — 2 MACs/PE/cycle,
~157 TF/s/core, 2x the f32r/bf16 rate. The fp8 quantization error
(std ~0.038*sigma) is absorbed by the host-side selection: a conservative
threshold screens candidates from the device f16 activations, the
(k*B)-th largest device value defines the cut, and everything within the
error band is recomputed exactly in fp32 from the original inputs so the
selected set matches an exact-fp32 reference.

The kernel returns scatter(top-(k*B) values) as a dense [B, D_DICT] fp32
array, matching the reference semantics (ties broken by lower flat index).
"""

import sys

sys.path.insert(0, "/opt/trn_rl_repo")

import numpy as np

# ---- problem constants (from the spec; asserted at runtime) ----
B = 2048           # batch
D = 2048           # activation dim (contraction)
DD = 32768         # dict size
NCORES = 8
FSH = DD // NCORES # 4096 dict rows per core
KT = D // 128      # 16 contraction tiles
FT = FSH // 128    # 32 f-tiles per core
NB = B // 512      # 4 batch chunks of 512
SW = 64.0          # pow2 weight scale so W*SW ~ N(0,1.28) fits e4m3 well

_STATE = {}


def _build_nc():
    from concourse import bacc
    import concourse.mybir as mybir
    import concourse.tile as tile

    F32 = mybir.dt.float32
    F16 = mybir.dt.float16
    F8 = mybir.dt.float8e4
    RELU = mybir.ActivationFunctionType.Relu
    DROW = mybir.MatmulPerfMode.DoubleRow

    nc = bacc.Bacc("TRN2", target_bir_lowering=False, debug=False, num_devices=NCORES)
    xt_d = nc.dram_tensor("xt", [128, NB, KT, 512], F8, kind="ExternalInput").ap()
    wt_d = nc.dram_tensor("wt", [128, FT, KT, 128], F8, kind="ExternalInput").ap()
    be_d = nc.dram_tensor("be", [128, FT], F32, kind="ExternalInput").ap()
    acts_d = nc.dram_tensor("acts", [FSH, B], F16, kind="ExternalOutput").ap()

    FA = 8  # phase-A f-tiles (resident W) — covers PE while x streams in

    with tile.TileContext(nc) as tc:
        with (
            tc.tile_pool(name="xres", bufs=1) as xpool,
            tc.tile_pool(name="wa", bufs=1) as wapool,
            tc.tile_pool(name="wstream", bufs=3) as wpool,
            tc.tile_pool(name="eplg", bufs=4) as opool,
            tc.tile_pool(name="ps", bufs=2, space="PSUM") as pspool,
        ):
            # DMA issue order matters: transfers complete in queue order, so
            # interleave the phase-A W tiles with the x chunks to unblock the
            # first matmul chains as early as possible.
            xts = [None] * NB
            was = [None] * FA

            def load_wa(f):
                wa = wapool.tile([128, KT, 128], F8, tag=f"wa{f}")
                nc.sync.dma_start(out=wa, in_=wt_d[:, f, :, :])
                was[f] = wa

            def load_x0(kks):
                # fine-grained per-kk slices of the first batch chunk: the
                # first chains start as soon as the first slices land
                if xts[0] is None:
                    xts[0] = xpool.tile([128, KT, 512], F8, tag="xt0", name="xt0")
                for kk in kks:
                    nc.sync.dma_start(
                        out=xts[0][:, kk : kk + 1, :], in_=xt_d[:, 0, kk : kk + 1, :]
                    )

            def load_xnb(nb):
                xnb = xpool.tile([128, KT, 512], F8, tag=f"xt{nb}")
                nc.sync.dma_start(out=xnb, in_=xt_d[:, nb, :, :])
                xts[nb] = xnb

            be = xpool.tile([128, FT], F32, tag="be")
            load_wa(0)
            load_x0(range(0, 4))
            load_wa(1)
            load_x0(range(4, 10))
            load_wa(2)
            load_x0(range(10, 16))
            nc.sync.dma_start(out=be, in_=be_d)
            load_wa(3)
            load_xnb(1)
            load_wa(4)
            load_wa(5)
            load_xnb(2)
            load_wa(6)
            load_wa(7)
            load_xnb(3)

            def chain(f, nb, wt):
                ps = pspool.tile([128, 512], F32, tag=f"ps{nb}")
                for kk in range(0, KT, 2):
                    nc.tensor.matmul(
                        ps,
                        wt[:, kk : kk + 2, :],
                        xts[nb][:, kk : kk + 2, :],
                        start=(kk == 0),
                        stop=(kk == KT - 2),
                        perf_mode=DROW,
                    )
                ot = opool.tile([128, 512], F16, tag="ot")
                nc.scalar.activation(
                    ot, ps, func=RELU, bias=be[:, f : f + 1], scale=1.0 / SW
                )
                nc.sync.dma_start(
                    out=acts_d[f * 128 : (f + 1) * 128, nb * 512 : (nb + 1) * 512],
                    in_=ot,
                )

            # phase A: nb-major staircase over the resident f-tiles, keeping
            # the PE busy while the rest of x is still loading
            for nb in range(NB):
                for f in range(FA):
                    chain(f, nb, was[f])

            # phase B: stream the remaining W tiles
            for f in range(FA, FT):
                wt = wpool.tile([128, KT, 128], F8, tag="wt")
                nc.sync.dma_start(out=wt, in_=wt_d[:, f, :, :])
                for nb in range(NB):
                    chain(f, nb, wt)

    nc.compile()
    return nc


def _get_nc():
    if "nc" not in _STATE:
        _STATE["nc"] = _build_nc()
    return _STATE["nc"]


def _fp8(a):
    import ml_dtypes

    return a.astype(ml_dtypes.float8_e4m3)


def _pack_x(xc):
    # xc [B, D] -> fp8 [128, NB, KT, 512]: (p, nb, kk, bb) = xc^T[kk*128+p, nb*512+bb]
    return np.ascontiguousarray(
        _fp8(xc).T.reshape(KT, 128, NB, 512).transpose(1, 2, 0, 3)
    )


def _pack_w(Wsh):
    # Wsh [FSH, D] -> fp8 [128, FT, KT, 128]: (p, f, kk, m) = (SW*W)[f*128+m, kk*128+p]
    return np.ascontiguousarray(
        _fp8(Wsh * np.float32(SW)).reshape(FT, 128, KT, 128).transpose(3, 0, 2, 1)
    )


def _get_runner():
    """Build the Bass program once and return a cached jitted SPMD callable.

    runner(xt, wt_concat, be_concat) -> actsT [DD, B] (numpy).
    xt is replicated to all 8 cores; wt/be are sharded along axis 0.
    """
    if "runner" in _STATE:
        return _STATE["runner"]

    import jax
    from jax.sharding import Mesh, PartitionSpec
    from jax.experimental.shard_map import shard_map
    from concourse import mybir
    from concourse.bass2jax import (
        _bass_exec_p,
        install_neuronx_cc_hook,
        partition_id_tensor,
    )

    nc = _get_nc()
    install_neuronx_cc_hook()

    pname = nc.partition_id_tensor.name if nc.partition_id_tensor else None
    in_names, out_names, out_avals = [], [], []
    for alloc in nc.m.functions[0].allocations:
        if not isinstance(alloc, mybir.MemoryLocationSet):
            continue
        name = alloc.memorylocations[0].name
        if alloc.kind == "ExternalInput":
            if name != pname:
                in_names.append(name)
        elif alloc.kind == "ExternalOutput":
            out_names.append(name)
            out_avals.append(
                jax.core.ShapedArray(tuple(alloc.tensor_shape), mybir.dt.np(alloc.dtype))
            )
    assert set(in_names) == {"xt", "wt", "be"}, in_names
    assert out_names == ["acts"], out_names
    all_in_names = in_names + out_names + ([pname] if pname else [])

    def _body(*args):
        operands = list(args)
        if pname:
            operands.append(partition_id_tensor())
        outs = _bass_exec_p.bind(
            *operands,
            out_avals=tuple(out_avals),
            in_names=tuple(all_in_names),
            out_names=tuple(out_names),
            lowering_input_output_aliases=(),
            sim_require_finite=True,
            sim_require_nnan=True,
            nc=nc,
        )
        return tuple(outs)

    devices = jax.devices()[:NCORES]
    assert len(devices) == NCORES, f"need {NCORES} neuron cores, got {len(devices)}"
    mesh = Mesh(np.asarray(devices), ("core",))
    arg_names = in_names + out_names
    in_specs = tuple(
        PartitionSpec() if nm == "xt" else PartitionSpec("core") for nm in arg_names
    )
    sharded = jax.jit(
        shard_map(
            _body,
            mesh=mesh,
            in_specs=in_specs,
            out_specs=(PartitionSpec("core"),),
            check_rep=False,
        )
    )

    from jax.sharding import NamedSharding

    # device-resident zero output-init buffers, uploaded once and reused
    zeros = [
        jax.device_put(
            np.zeros((NCORES * a.shape[0], *a.shape[1:]), a.dtype),
            NamedSharding(mesh, PartitionSpec("core")),
        )
        for a in out_avals
    ]

    def runner(xt, wt_concat, be_concat):
        args = {"xt": xt, "wt": wt_concat, "be": be_concat}
        out = sharded(*[args[nm] for nm in in_names], *zeros)
        return np.asarray(out[0])  # [DD, B]

    _STATE["runner"] = runner
    return runner


def _prep_inputs(x, W_enc, b_enc, b_dec):
    xc = (x.astype(np.float32) - b_dec.astype(np.float32)[None, :]).astype(np.float32)
    xt = _pack_x(xc)
    wt_concat = np.concatenate(
        [
            _pack_w(np.ascontiguousarray(W_enc[c * FSH : (c + 1) * FSH], dtype=np.float32))
            for c in range(NCORES)
        ],
        axis=0,
    )
    be_concat = np.concatenate(
        [
            np.ascontiguousarray(
                b_enc[c * FSH : (c + 1) * FSH].astype(np.float32).reshape(FT, 128).T
            )
            for c in range(NCORES)
        ],
        axis=0,
    )
    return xt, wt_concat, be_concat


def _run_device(x, W_enc, b_enc, b_dec, trace=False, trace_kwargs=None):
    if trace:
        # profiling path via run_bass_kernel_spmd (NTFF capture)
        from concourse.bass_utils import run_bass_kernel_spmd

        nc = _get_nc()
        xc = (x.astype(np.float32) - b_dec.astype(np.float32)[None, :]).astype(
            np.float32
        )
        xt = _pack_x(xc)
        in_maps = []
        for c in range(NCORES):
            in_maps.append(
                {
                    "xt": xt,
                    "wt": _pack_w(
                        np.ascontiguousarray(
                            W_enc[c * FSH : (c + 1) * FSH], dtype=np.float32
                        )
                    ),
                    "be": np.ascontiguousarray(
                        b_enc[c * FSH : (c + 1) * FSH]
                        .astype(np.float32)
                        .reshape(FT, 128)
                        .T
                    ),
                }
            )
        res = run_bass_kernel_spmd(
            nc, in_maps, list(range(NCORES)), trace=True, **(trace_kwargs or {})
        )
        _STATE["last_result"] = res
        return np.concatenate(
            [res.results[c]["acts"] for c in range(NCORES)], axis=0
        )

    runner = _get_runner()
    xt, wt_concat, be_concat = _prep_inputs(x, W_enc, b_enc, b_dec)
    return runner(xt, wt_concat, be_concat)


def _exact_vals(x32, W32, be64, f_idx, b_idx):
    """Accurate fp32 recompute of pre-relu acts at (b, f) pairs.

    Grouped by batch row so each group is a single BLAS sgemv — same
    accuracy class as the reference's own fp32 einsum.
    """
    n = len(f_idx)
    if n == 0:
        return np.zeros(0, np.float64)
    order = np.argsort(b_idx, kind="stable")
    fs, bs = f_idx[order], b_idx[order]
    ub, starts = np.unique(bs, return_index=True)
    ends = np.append(starts[1:], n)
    out = np.empty(n, np.float32)
    for i, b in enumerate(ub):
        s, e = starts[i], ends[i]
        out[s:e] = W32[fs[s:e]] @ x32[b]
    res = np.empty(n, np.float64)
    res[order] = out.astype(np.float64)
    return res + be64[f_idx]


def _select_topk(actsT, kb, x32, W32, be64, sigma):
    """Exact top-kb selection (reference semantics) from device f16 acts.

    Returns (b_idx, f_idx, values[fp32]) of the selected elements.
    actsT: [DD, B] float16 device activations.
    """
    DDl, Bl = actsT.shape
    total = DDl * Bl
    empty = (np.zeros(0, np.int64), np.zeros(0, np.int64), np.zeros(0, np.float32))
    if kb <= 0:
        return empty
    kb = min(kb, total)

    # abs error bound of device f16 acts vs exact fp32: fp8 e4m3 quantization
    # of both GEMM operands gives err std ~0.038*sigma; 0.25*sigma is a
    # ~6.5-std bound (f16 storage quantization is negligible next to it)
    errtot = max(0.25 * sigma, 1e-7)

    # conservative screen: comfortably more candidates than kb
    cnt = 0
    for t_frac in (2.45, 2.0, 1.5, 1.0, 0.5, 0.0):
        t_lo = t_frac * sigma
        m = actsT > np.float16(t_lo)
        cnt = int(m.sum())
        if cnt >= kb + max(1024, kb // 16) or t_frac == 0.0:
            break

    f_idx, b_idx = np.nonzero(m)
    vals = actsT[m].astype(np.float32)

    if cnt <= kb:
        # everything positive is selected (selected zeros are no-ops)
        ex = _exact_vals(x32, W32, be64, f_idx, b_idx)
        keep = ex > 0
        return (
            b_idx[keep],
            f_idx[keep],
            np.maximum(ex[keep], 0.0).astype(np.float32),
        )

    part = np.partition(vals, cnt - kb)
    tau_dev = float(part[cnt - kb])

    band = 2.5 * errtot
    for _ in range(24):
        refine = vals > tau_dev - band
        nr = int(refine.sum())
        if nr < kb:
            band *= 2.0
            continue
        fr, br = f_idx[refine], b_idx[refine]
        ex = _exact_vals(x32, W32, be64, fr, br)
        flat = br.astype(np.int64) * DDl + fr.astype(np.int64)
        # reference order: value desc, flat index asc on ties
        order = np.lexsort((flat, -ex))
        take = order[:kb]
        tau_exact = float(ex[take[-1]])
        # excluded elements either have f16 <= tau_dev - band or were below
        # the screen threshold t_lo, so their exact value is at most
        # max(tau_dev - band, t_lo) + errtot; selection is airtight iff
        # tau_exact is above that.
        excl_hi = max(tau_dev - band, t_lo) + errtot
        if tau_exact > excl_hi or (band > 2.0 * sigma + 1.0):
            vsel = np.maximum(ex[take], 0.0).astype(np.float32)
            return (br[take], fr[take], vsel)
        band *= 2.0
        if (tau_dev - band < t_lo + errtot or tau_exact <= t_lo + errtot) and t_lo > 0:
            # widen past the screen: fall back to all-positives screen
            m = actsT > np.float16(0.0)
            cnt = int(m.sum())
            f_idx, b_idx = np.nonzero(m)
            vals = actsT[m].astype(np.float32)
            t_lo = 0.0
            if cnt <= kb:
                ex = _exact_vals(x32, W32, be64, f_idx, b_idx)
                keep = ex > 0
                return (
                    b_idx[keep],
                    f_idx[keep],
                    np.maximum(ex[keep], 0.0).astype(np.float32),
                )
            part = np.partition(vals, cnt - kb)
            tau_dev = float(part[cnt - kb])
    raise RuntimeError("top-k band search failed to converge")


def _kernel_numpy_fallback(x, W_enc, b_enc, b_dec, k):
    x32 = x.astype(np.float32)
    acts = np.maximum(
        (x32 - b_dec.astype(np.float32)) @ W_enc.astype(np.float32).T
        + b_enc.astype(np.float32),
        0.0,
    )
    flat = acts.reshape(-1)
    kb = int(k) * x.shape[0]
    if kb <= 0:
        return np.zeros_like(acts)
    kb = min(kb, flat.size)
    idx = np.argpartition(flat, flat.size - kb)[flat.size - kb :]
    # exact reference tie-break: value desc, index asc
    order = np.lexsort((idx, -flat[idx].astype(np.float64)))
    idx = idx[order[:kb]]
    out = np.zeros_like(flat)
    out[idx] = flat[idx]
    return out.reshape(acts.shape)


def kernel(x, W_enc, b_enc, b_dec, k):
    x = np.asarray(x)
    W_enc = np.asarray(W_enc)
    b_enc = np.asarray(b_enc)
    b_dec = np.asarray(b_dec)
    kb = int(k) * x.shape[0]

    if x.shape != (B, D) or W_enc.shape != (DD, D):
        return _kernel_numpy_fallback(x, W_enc, b_enc, b_dec, k)

    actsT = _run_device(x, W_enc, b_enc, b_dec)  # [DD, B] f16

    if not np.all(np.isfinite(actsT[:: max(1, DD // 256)])) or np.any(
        actsT[:: max(1, DD // 256)] == np.inf
    ):
        return _kernel_numpy_fallback(x, W_enc, b_enc, b_dec, k)

    x32 = (x.astype(np.float32) - b_dec.astype(np.float32)[None, :]).astype(np.float32)
    W32 = np.ascontiguousarray(W_enc.astype(np.float32))
    be64 = b_enc.astype(np.float64)

    sub = actsT[:: max(1, DD // 1024)].astype(np.float32)
    sigma = float(np.sqrt(2.0 * np.mean(np.square(sub))))
    if not np.isfinite(sigma) or sigma <= 0:
        sigma = 1.0

    b_sel, f_sel, v_sel = _select_topk(actsT, kb, x32, W32, be64, sigma)

    out = np.zeros((B, DD), np.float32)
    out[b_sel, f_sel] = v_sel
    return out


# revision 5
# speedup vs baseline: 2.3683x; 1.0320x over previous
"""BatchTopK SAE encoder on 8 Trainium2 NeuronCores (fp8 DoubleRow GEMM).

Strategy
--------
Tensor-parallel over dict_size: core c computes the encoder GEMM for dict
rows [c*4096, (c+1)*4096):

    acts_c^T [4096, 2048] = relu((W_c*64)_fp8 @ (x - b_dec)_fp8^T / 64 + b_enc_c)

as float8e4 (e4m3) matmuls in MatmulPerfMode.DoubleRow — 2 MACs/PE/cycle,
~157 TF/s/core, 2x the f32r/bf16 rate. The fp8 quantization error
(std ~0.038*sigma) is absorbed by the host-side selection: a conservative
threshold screens candidates from the device f16 activations, the
(k*B)-th largest device value defines the cut, and everything within the
error band is recomputed exactly in fp32 from the original inputs so the
selected set matches an exact-fp32 reference.

The kernel returns scatter(top-(k*B) values) as a dense [B, D_DICT] fp32
array, matching the reference semantics (ties broken by lower flat index).
"""

import sys

sys.path.insert(0, "/opt/trn_rl_repo")

import numpy as np

# ---- problem constants (from the spec; asserted at runtime) ----
B = 2048           # batch
D = 2048           # activation dim (contraction)
DD = 32768         # dict size
NCORES = 8
FSH = DD // NCORES # 4096 dict rows per core
KT = D // 128      # 16 contraction tiles
FT = FSH // 128    # 32 f-tiles per core
NB = B // 512      # 4 batch chunks of 512
SW = 64.0          # pow2 weight scale so W*SW ~ N(0,1.28) fits e4m3 well

_STATE = {}


def _build_nc():
    from concourse import bacc
    import concourse.mybir as mybir
    import concourse.tile as tile

    F32 = mybir.dt.float32
    F16 = mybir.dt.float16
    F8 = mybir.dt.float8e4
    RELU = mybir.ActivationFunctionType.Relu
    DROW = mybir.MatmulPerfMode.DoubleRow

    nc = bacc.Bacc("TRN2", target_bir_lowering=False, debug=False, num_devices=NCORES)
    xt_d = nc.dram_tensor("xt", [128, NB, KT, 512], F8, kind="ExternalInput").ap()
    wt_d = nc.dram_tensor("wt", [128, FT, KT, 128], F8, kind="ExternalInput").ap()
    be_d = nc.dram_tensor("be", [128, FT], F32, kind="ExternalInput").ap()
    acts_d = nc.dram_tensor("acts", [FSH, B], F16, kind="ExternalOutput").ap()

    FA = 8  # phase-A f-tiles (resident W) — covers PE while x streams in

    with tile.TileContext(nc) as tc:
        with (
            tc.tile_pool(name="xres", bufs=1) as xpool,
            tc.tile_pool(name="wa", bufs=1) as wapool,
            tc.tile_pool(name="wstream", bufs=3) as wpool,
            tc.tile_pool(name="eplg", bufs=4) as opool,
            tc.tile_pool(name="ps", bufs=2, space="PSUM") as pspool,
        ):
            # DMA issue order matters: transfers complete in queue order.
            # Priority: wa0 + the nb0 x pair-tiles (to start the first chain
            # ASAP), then the remaining phase-A W tiles (consumed at 1.7us
            # per chain), and only then the x chunks for nb1-3 (not needed
            # until ~14us+).
            xts = [None] * NB
            x0p = [None] * (KT // 2)  # nb0 as independent DoubleRow pair-tiles
            was = [None] * FA

            def load_wa(f):
                wa = wapool.tile([128, KT, 128], F8, tag=f"wa{f}")
                nc.sync.dma_start(out=wa, in_=wt_d[:, f, :, :])
                was[f] = wa

            def load_x0p(js):
                for j in js:
                    xp = xpool.tile([128, 2, 512], F8, tag=f"xp{j}", name=f"xp{j}")
                    nc.sync.dma_start(
                        out=xp, in_=xt_d[:, 0, 2 * j : 2 * j + 2, :]
                    )
                    x0p[j] = xp

            def load_xnb(nb):
                xnb = xpool.tile([128, KT, 512], F8, tag=f"xt{nb}")
                nc.sync.dma_start(out=xnb, in_=xt_d[:, nb, :, :])
                xts[nb] = xnb

            be = xpool.tile([128, FT], F32, tag="be")
            load_wa(0)
            load_x0p(range(0, 3))
            load_wa(1)
            load_x0p(range(3, 8))
            nc.sync.dma_start(out=be, in_=be_d)
            load_wa(2)
            load_wa(3)
            load_wa(4)
            load_wa(5)
            load_wa(6)
            load_wa(7)
            load_xnb(1)
            load_xnb(2)
            load_xnb(3)

            chain_no = [0]

            def chain(f, nb, wt):
                # rotate psum tags globally: 4 tags x 2 bufs = 8 banks, so a
                # bank is only reused 8 chains later — the epilogue has ~13us
                # of slack instead of gating the PE after 2 chains
                ps = pspool.tile([128, 512], F32, tag=f"ps{chain_no[0] % 4}")
                chain_no[0] += 1
                for kk in range(0, KT, 2):
                    rhs = x0p[kk // 2] if nb == 0 else xts[nb][:, kk : kk + 2, :]
                    nc.tensor.matmul(
                        ps,
                        wt[:, kk : kk + 2, :],
                        rhs,
                        start=(kk == 0),
                        stop=(kk == KT - 2),
                        perf_mode=DROW,
                    )
                ot = opool.tile([128, 512], F16, tag="ot")
                nc.scalar.activation(
                    ot, ps, func=RELU, bias=be[:, f : f + 1], scale=1.0 / SW
                )
                nc.sync.dma_start(
                    out=acts_d[f * 128 : (f + 1) * 128, nb * 512 : (nb + 1) * 512],
                    in_=ot,
                )

            # phase A: nb-major staircase over the resident f-tiles, keeping
            # the PE busy while the rest of x is still loading
            for nb in range(NB):
                for f in range(FA):
                    chain(f, nb, was[f])

            # phase B: stream the remaining W tiles
            for f in range(FA, FT):
                wt = wpool.tile([128, KT, 128], F8, tag="wt")
                nc.sync.dma_start(out=wt, in_=wt_d[:, f, :, :])
                for nb in range(NB):
                    chain(f, nb, wt)

    nc.compile()
    return nc


def _get_nc():
    if "nc" not in _STATE:
        _STATE["nc"] = _build_nc()
    return _STATE["nc"]


def _fp8(a):
    import ml_dtypes

    return a.astype(ml_dtypes.float8_e4m3)


def _pack_x(xc):
    # xc [B, D] -> fp8 [128, NB, KT, 512]: (p, nb, kk, bb) = xc^T[kk*128+p, nb*512+bb]
    return np.ascontiguousarray(
        _fp8(xc).T.reshape(KT, 128, NB, 512).transpose(1, 2, 0, 3)
    )


def _pack_w(Wsh):
    # Wsh [FSH, D] -> fp8 [128, FT, KT, 128]: (p, f, kk, m) = (SW*W)[f*128+m, kk*128+p]
    return np.ascontiguousarray(
        _fp8(Wsh * np.float32(SW)).reshape(FT, 128, KT, 128).transpose(3, 0, 2, 1)
    )


def _get_runner():
    """Build the Bass program once and return a cached jitted SPMD callable.

    runner(xt, wt_concat, be_concat) -> actsT [DD, B] (numpy).
    xt is replicated to all 8 cores; wt/be are sharded along axis 0.
    """
    if "runner" in _STATE:
        return _STATE["runner"]

    import jax
    from jax.sharding import Mesh, PartitionSpec
    from jax.experimental.shard_map import shard_map
    from concourse import mybir
    from concourse.bass2jax import (
        _bass_exec_p,
        install_neuronx_cc_hook,
        partition_id_tensor,
    )

    nc = _get_nc()
    install_neuronx_cc_hook()

    pname = nc.partition_id_tensor.name if nc.partition_id_tensor else None
    in_names, out_names, out_avals = [], [], []
    for alloc in nc.m.functions[0].allocations:
        if not isinstance(alloc, mybir.MemoryLocationSet):
            continue
        name = alloc.memorylocations[0].name
        if alloc.kind == "ExternalInput":
            if name != pname:
                in_names.append(name)
        elif alloc.kind == "ExternalOutput":
            out_names.append(name)
            out_avals.append(
                jax.core.ShapedArray(tuple(alloc.tensor_shape), mybir.dt.np(alloc.dtype))
            )
    assert set(in_names) == {"xt", "wt", "be"}, in_names
    assert out_names == ["acts"], out_names
    all_in_names = in_names + out_names + ([pname] if pname else [])

    def _body(*args):
        operands = list(args)
        if pname:
            operands.append(partition_id_tensor())
        outs = _bass_exec_p.bind(
            *operands,
            out_avals=tuple(out_avals),
            in_names=tuple(all_in_names),
            out_names=tuple(out_names),
            lowering_input_output_aliases=(),
            sim_require_finite=True,
            sim_require_nnan=True,
            nc=nc,
        )
        return tuple(outs)

    devices = jax.devices()[:NCORES]
    assert len(devices) == NCORES, f"need {NCORES} neuron cores, got {len(devices)}"
    mesh = Mesh(np.asarray(devices), ("core",))
    arg_names = in_names + out_names
    in_specs = tuple(
        PartitionSpec() if nm == "xt" else PartitionSpec("core") for nm in arg_names
    )
    sharded = jax.jit(
        shard_map(
            _body,
            mesh=mesh,
            in_specs=in_specs,
            out_specs=(PartitionSpec("core"),),
            check_rep=False,
        )
    )

    from jax.sharding import NamedSharding

    # device-resident zero output-init buffers, uploaded once and reused
    zeros = [
        jax.device_put(
            np.zeros((NCORES * a.shape[0], *a.shape[1:]), a.dtype),
            NamedSharding(mesh, PartitionSpec("core")),
        )
        for a in out_avals
    ]

    def runner(xt, wt_concat, be_concat):
        args = {"xt": xt, "wt": wt_concat, "be": be_concat}
        out = sharded(*[args[nm] for nm in in_names], *zeros)
        return np.asarray(out[0])  # [DD, B]

    _STATE["runner"] = runner
    return runner


def _prep_inputs(x, W_enc, b_enc, b_dec):
    xc = (x.astype(np.float32) - b_dec.astype(np.float32)[None, :]).astype(np.float32)
    xt = _pack_x(xc)
    wt_concat = np.concatenate(
        [
            _pack_w(np.ascontiguousarray(W_enc[c * FSH : (c + 1) * FSH], dtype=np.float32))
            for c in range(NCORES)
        ],
        axis=0,
    )
    be_concat = np.concatenate(
        [
            np.ascontiguousarray(
                b_enc[c * FSH : (c + 1) * FSH].astype(np.float32).reshape(FT, 128).T
            )
            for c in range(NCORES)
        ],
        axis=0,
    )
    return xt, wt_concat, be_concat


def _run_device(x, W_enc, b_enc, b_dec, trace=False, trace_kwargs=None):
    if trace:
        # profiling path via run_bass_kernel_spmd (NTFF capture)
        from concourse.bass_utils import run_bass_kernel_spmd

        nc = _get_nc()
        xc = (x.astype(np.float32) - b_dec.astype(np.float32)[None, :]).astype(
            np.float32
        )
        xt = _pack_x(xc)
        in_maps = []
        for c in range(NCORES):
            in_maps.append(
                {
                    "xt": xt,
                    "wt": _pack_w(
                        np.ascontiguousarray(
                            W_enc[c * FSH : (c + 1) * FSH], dtype=np.float32
                        )
                    ),
                    "be": np.ascontiguousarray(
                        b_enc[c * FSH : (c + 1) * FSH]
                        .astype(np.float32)
                        .reshape(FT, 128)
                        .T
                    ),
                }
            )
        res = run_bass_kernel_spmd(
            nc, in_maps, list(range(NCORES)), trace=True, **(trace_kwargs or {})
        )
        _STATE["last_result"] = res
        return np.concatenate(
            [res.results[c]["acts"] for c in range(NCORES)], axis=0
        )

    runner = _get_runner()
    xt, wt_concat, be_concat = _prep_inputs(x, W_enc, b_enc, b_dec)
    return runner(xt, wt_concat, be_concat)


def _exact_vals(x32, W32, be64, f_idx, b_idx):
    """Accurate fp32 recompute of pre-relu acts at (b, f) pairs.

    Grouped by batch row so each group is a single BLAS sgemv — same
    accuracy class as the reference's own fp32 einsum.
    """
    n = len(f_idx)
    if n == 0:
        return np.zeros(0, np.float64)
    order = np.argsort(b_idx, kind="stable")
    fs, bs = f_idx[order], b_idx[order]
    ub, starts = np.unique(bs, return_index=True)
    ends = np.append(starts[1:], n)
    out = np.empty(n, np.float32)
    for i, b in enumerate(ub):
        s, e = starts[i], ends[i]
        out[s:e] = W32[fs[s:e]] @ x32[b]
    res = np.empty(n, np.float64)
    res[order] = out.astype(np.float64)
    return res + be64[f_idx]


def _select_topk(actsT, kb, x32, W32, be64, sigma):
    """Exact top-kb selection (reference semantics) from device f16 acts.

    Returns (b_idx, f_idx, values[fp32]) of the selected elements.
    actsT: [DD, B] float16 device activations.
    """
    DDl, Bl = actsT.shape
    total = DDl * Bl
    empty = (np.zeros(0, np.int64), np.zeros(0, np.int64), np.zeros(0, np.float32))
    if kb <= 0:
        return empty
    kb = min(kb, total)

    # abs error bound of device f16 acts vs exact fp32: fp8 e4m3 quantization
    # of both GEMM operands gives err std ~0.038*sigma; 0.25*sigma is a
    # ~6.5-std bound (f16 storage quantization is negligible next to it)
    errtot = max(0.25 * sigma, 1e-7)

    # conservative screen: comfortably more candidates than kb
    cnt = 0
    for t_frac in (2.45, 2.0, 1.5, 1.0, 0.5, 0.0):
        t_lo = t_frac * sigma
        m = actsT > np.float16(t_lo)
        cnt = int(m.sum())
        if cnt >= kb + max(1024, kb // 16) or t_frac == 0.0:
            break

    f_idx, b_idx = np.nonzero(m)
    vals = actsT[m].astype(np.float32)

    if cnt <= kb:
        # everything positive is selected (selected zeros are no-ops)
        ex = _exact_vals(x32, W32, be64, f_idx, b_idx)
        keep = ex > 0
        return (
            b_idx[keep],
            f_idx[keep],
            np.maximum(ex[keep], 0.0).astype(np.float32),
        )

    part = np.partition(vals, cnt - kb)
    tau_dev = float(part[cnt - kb])

    band = 2.5 * errtot
    for _ in range(24):
        refine = vals > tau_dev - band
        nr = int(refine.sum())
        if nr < kb:
            band *= 2.0
            continue
        fr, br = f_idx[refine], b_idx[refine]
        ex = _exact_vals(x32, W32, be64, fr, br)
        flat = br.astype(np.int64) * DDl + fr.astype(np.int64)
        # reference order: value desc, flat index asc on ties
        order = np.lexsort((flat, -ex))
        take = order[:kb]
        tau_exact = float(ex[take[-1]])
        # excluded elements either have f16 <= tau_dev - band or were below
        # the screen threshold t_lo, so their exact value is at most
        # max(tau_dev - band, t_lo) + errtot; selection is airtight iff
        # tau_exact is above that.
        excl_hi = max(tau_dev - band, t_lo) + errtot
        if tau_exact > excl_hi or (band > 2.0 * sigma + 1.0):
            vsel = np.maximum(ex[take], 0.0).astype(np.float32)
            return (br[take], fr[take], vsel)
        band *= 2.0
        if (tau_dev - band < t_lo + errtot or tau_exact <= t_lo + errtot) and t_lo > 0:
            # widen past the screen: fall back to all-positives screen
            m = actsT > np.float16(0.0)
            cnt = int(m.sum())
            f_idx, b_idx = np.nonzero(m)
            vals = actsT[m].astype(np.float32)
            t_lo = 0.0
            if cnt <= kb:
                ex = _exact_vals(x32, W32, be64, f_idx, b_idx)
                keep = ex > 0
                return (
                    b_idx[keep],
                    f_idx[keep],
                    np.maximum(ex[keep], 0.0).astype(np.float32),
                )
            part = np.partition(vals, cnt - kb)
            tau_dev = float(part[cnt - kb])
    raise RuntimeError("top-k band search failed to converge")


def _kernel_numpy_fallback(x, W_enc, b_enc, b_dec, k):
    x32 = x.astype(np.float32)
    acts = np.maximum(
        (x32 - b_dec.astype(np.float32)) @ W_enc.astype(np.float32).T
        + b_enc.astype(np.float32),
        0.0,
    )
    flat = acts.reshape(-1)
    kb = int(k) * x.shape[0]
    if kb <= 0:
        return np.zeros_like(acts)
    kb = min(kb, flat.size)
    idx = np.argpartition(flat, flat.size - kb)[flat.size - kb :]
    # exact reference tie-break: value desc, index asc
    order = np.lexsort((idx, -flat[idx].astype(np.float64)))
    idx = idx[order[:kb]]
    out = np.zeros_like(flat)
    out[idx] = flat[idx]
    return out.reshape(acts.shape)


def kernel(x, W_enc, b_enc, b_dec, k):
    x = np.asarray(x)
    W_enc = np.asarray(W_enc)
    b_enc = np.asarray(b_enc)
    b_dec = np.asarray(b_dec)
    kb = int(k) * x.shape[0]

    if x.shape != (B, D) or W_enc.shape != (DD, D):
        return _kernel_numpy_fallback(x, W_enc, b_enc, b_dec, k)

    actsT = _run_device(x, W_enc, b_enc, b_dec)  # [DD, B] f16

    if not np.all(np.isfinite(actsT[:: max(1, DD // 256)])) or np.any(
        actsT[:: max(1, DD // 256)] == np.inf
    ):
        return _kernel_numpy_fallback(x, W_enc, b_enc, b_dec, k)

    x32 = (x.astype(np.float32) - b_dec.astype(np.float32)[None, :]).astype(np.float32)
    W32 = np.ascontiguousarray(W_enc.astype(np.float32))
    be64 = b_enc.astype(np.float64)

    sub = actsT[:: max(1, DD // 1024)].astype(np.float32)
    sigma = float(np.sqrt(2.0 * np.mean(np.square(sub))))
    if not np.isfinite(sigma) or sigma <= 0:
        sigma = 1.0

    b_sel, f_sel, v_sel = _select_topk(actsT, kb, x32, W32, be64, sigma)

    out = np.zeros((B, DD), np.float32)
    out[b_sel, f_sel] = v_sel
    return out
